# revision 1
# baseline (speedup 1.0000x reference)
"""GNN message-passing encoder (nn_Encoder) for 8 Trainium2 NeuronCores.

Contract: kernel(**inputs) takes the FULL unsharded inputs of
reference.setup_inputs() and returns the FULL [1024, 8, 8, 384] float32
output. Internally the node batch (B*L = 8192 flat nodes) is sharded
1024 nodes per core across 8 cores; the feature table, neighbor lists
and weights are replicated.

Per-core device program (node n = block*128 + p, 8 blocks):
  - features cast to fp16; rows fetched with InstDMAGatherAnt (the fast
    SWDGE gather) in 4 table segments of 25000 rows so local indices fit
    int16; one SWDGE queue per segment (4-way parallel descriptor gen).
  - gather lists are built on the host per (block, segment), rows sorted
    by node; rows land [128 part, chunk, 128] fp16.
  - reduction (mean over 10 hop-1 / 100 hop-2 rows): per 128-row chunk a
    matmul lhsT=G_c [row, d], rhs=M_c [row, node-window] with 0/1
    selection weights accumulates feats^T [d, node] in PSUM.
  - projection: feats^T (fp32) x W^T [d, clips*dim] on the PE, then
    fused ReLU+mean-scale on the scalar engine, one 1.5 MB store per
    block.
"""
import numpy as np

P = 128
NBLK = 8
S1, S2 = 10, 10
D = 128
CK = 1024
V = 100000
NSEG = 4
SEG = V // NSEG

BR_HOP1, BR_HOP2, BR_SELF = 0, 1, 2
BR_SCALE = {BR_HOP1: 0.1, BR_HOP2: 0.01, BR_SELF: 1.0}
BR_SEG = {BR_HOP1: 0, BR_HOP2: 1, BR_SELF: 2}


def _host_prep(nodes, neigh1, neigh2, core):
    nf = np.asarray(nodes).reshape(-1)
    shard = nf[core * 1024:(core + 1) * 1024].astype(np.int64)
    n1 = np.asarray(neigh1)[shard]
    n2 = np.asarray(neigh2)[shard]
    th = np.asarray(neigh1)[n2]

    rows_by_branch = {
        BR_HOP1: (np.repeat(np.arange(1024), S1), n1.reshape(-1).astype(np.int64)),
        BR_HOP2: (np.repeat(np.arange(1024), S1 * S2), th.reshape(-1).astype(np.int64)),
        BR_SELF: (np.arange(1024), shard),
    }
    br_order = [BR_HOP2, BR_HOP1, BR_SELF]

    calls = []
    for b in range(NBLK):
        nlo, nhi = b * 128, (b + 1) * 128
        for s in range(NSEG):
            idx_list, node_list, br_bounds = [], [], []
            for br in br_order:
                nd, rw = rows_by_branch[br]
                m = (nd >= nlo) & (nd < nhi) & (rw // SEG == s)
                nd_s, rw_s = nd[m], rw[m]
                order = np.argsort(nd_s, kind="stable")
                start = len(idx_list)
                idx_list.extend((rw_s[order] - s * SEG).tolist())
                node_list.extend((nd_s[order] - nlo).tolist())
                br_bounds.append((br, start, len(idx_list)))
            n_valid = len(idx_list)
            n_pad = (-n_valid) % P
            # pad with a valid index (0) so every dst row is written
            # (garbage rows would be NaN-unsafe even with 0 M-weight)
            padded = np.array(idx_list + [0] * n_pad, dtype=np.int16)
            nodecol = np.array(node_list + [0] * n_pad, dtype=np.int64)
            chunks = []
            for c in range(len(padded) // P):
                lo, hi = c * P, (c + 1) * P
                for (br, bs, be) in br_bounds:
                    a, z = max(lo, bs), min(hi, be)
                    if a >= z:
                        continue
                    cols = nodecol[a:z]
                    col0 = int(cols.min())
                    w = int(cols.max()) - col0 + 1
                    M = np.zeros((P, w), dtype=np.float16)
                    M[np.arange(a, z) - lo, cols - col0] = 1.0
                    chunks.append((c, br, col0, w, M))
            calls.append({"block": b, "seg": s, "idx": padded,
                          "n_valid": len(padded), "chunks": chunks})
    return calls


def _wrap_idx(idx):
    N = len(idx)
    w16 = idx.reshape(N // 16, 16).T.astype(np.int16)
    return np.tile(w16, (8, 1))


def _pack_core(features16, WT, nodes, neigh1, neigh2, core):
    calls = _host_prep(nodes, neigh1, neigh2, core)
    idx_parts, idx_off = [], []
    off = 0
    for cl in calls:
        w = _wrap_idx(cl["idx"])
        idx_parts.append(w)
        idx_off.append((off, w.shape[1], len(cl["idx"]), cl["n_valid"]))
        off += w.shape[1]
    idx_all = np.concatenate(idx_parts, axis=1)
    m_parts, m_meta = [], []
    moff = 0
    for cl in calls:
        lst = []
        for (c, br, col0, w, M) in cl["chunks"]:
            lst.append((c, br, col0, w, moff))
            m_parts.append(M)
            moff += w
        m_meta.append(lst)
    m_all = np.concatenate(m_parts, axis=1)

    meta = {
        "calls": [{"block": cl["block"], "seg": cl["seg"],
                   "idx_off": idx_off[ci], "chunks": m_meta[ci]}
                  for ci, cl in enumerate(calls)],
        "idx_cols": idx_all.shape[1],
        "m_cols": m_all.shape[1],
    }
    in_map = {"feat": features16, "wt": WT,
              "idxs": np.ascontiguousarray(idx_all),
              "mmat": np.ascontiguousarray(m_all)}
    return in_map, meta


def build_core_program(meta):
    import concourse.bacc as bacc
    import concourse.mybir as mybir
    from concourse.tile import TileContext
    from concourse.library_config import mlp

    f16, f32, i16 = mybir.dt.float16, mybir.dt.float32, mybir.dt.int16

    nc = bacc.Bacc(num_swdge_queues=4)
    feat = nc.declare_dram_parameter("feat", [V, D], f16, isOutput=False)
    wt = nc.declare_dram_parameter("wt", [D, CK], f32, isOutput=False)
    idxs = nc.declare_dram_parameter("idxs", [P, meta["idx_cols"]], i16, isOutput=False)
    mmat = nc.declare_dram_parameter("mmat", [P, meta["m_cols"]], f16, isOutput=False)
    out = nc.declare_dram_parameter("out", [NBLK, P, 3 * CK], f16, isOutput=True)

    by_block = {}
    for cl in meta["calls"]:
        by_block.setdefault(cl["block"], []).append(cl)

    with TileContext(nc) as tc:
        with (
            tc.tile_pool(name="const", bufs=1) as constp,
            tc.tile_pool(name="idxp", bufs=8) as idxp,
            tc.tile_pool(name="mp", bufs=8) as mp,
            tc.tile_pool(name="dstp", bufs=2) as dstp,
            tc.tile_pool(name="ftp", bufs=3) as ftp,
            tc.tile_pool(name="stp", bufs=2) as stp,
            tc.tile_pool(name="ps_red", bufs=2, space="PSUM") as ps_red,
            tc.tile_pool(name="ps_mm", bufs=2, space="PSUM") as ps_mm,
        ):
            nc.gpsimd.load_library(mlp)
            wt_t = constp.tile([P, CK], f32, tag="wt")
            nc.sync.dma_start(out=wt_t[:], in_=wt[:])
            zrhs = constp.tile([P, P], f16, tag="zrhs")
            nc.vector.memset(zrhs[:], 0.0)

            for b in range(NBLK):
                cls = by_block[b]
                i0 = min(cl["idx_off"][0] for cl in cls)
                i1 = max(cl["idx_off"][0] + cl["idx_off"][1] for cl in cls)
                it = idxp.tile([P, i1 - i0], i16, tag="it")
                nc.sync.dma_start(out=it[:], in_=idxs[:, i0:i1])
                moffs = [mo for cl in cls for (_, _, _, _, mo) in cl["chunks"]]
                mws = [w for cl in cls for (_, _, _, w, _) in cl["chunks"]]
                m0 = min(moffs)
                m1 = max(mo + w for mo, w in zip(moffs, mws))
                mt = mp.tile([P, m1 - m0], f16, tag="mt")
                nc.sync.dma_start(out=mt[:], in_=mmat[:, m0:m1])

                dsts = {}
                for cl in cls:
                    off, wcols, n_padded, n_valid = cl["idx_off"]
                    if n_valid == 0:
                        continue
                    nch = n_padded // P
                    dtile = dstp.tile([P, nch, D], f16, tag=f"dst{cl['seg']}")
                    nc.gpsimd.dma_gather(
                        dtile[:], feat[cl["seg"] * SEG:(cl["seg"] + 1) * SEG, :],
                        it[:, off - i0:off - i0 + wcols],
                        n_padded, n_valid, D,
                        single_packet=False, queue_num=cl["seg"])
                    dsts[cl["seg"]] = dtile

                reds = []
                for br in range(3):
                    rt = ps_red.tile([P, P], f32, tag=f"red{br}", space="PSUM")
                    nc.tensor.matmul(out=rt[:], lhsT=zrhs[:], rhs=zrhs[:],
                                     start=True, stop=False, skip_group_check=True)
                    reds.append(rt)
                for cl in cls:
                    if cl["idx_off"][3] == 0:
                        continue
                    dtile = dsts[cl["seg"]]
                    for (c, br, col0, w, mo) in cl["chunks"]:
                        nc.tensor.matmul(
                            out=reds[br][:, col0:col0 + w],
                            lhsT=dtile[:, c, :],
                            rhs=mt[:, mo - m0:mo - m0 + w],
                            start=False, stop=False, skip_group_check=True)

                stage = stp.tile([P, 8, 3, D], f16, tag="stage")
                for br in range(3):
                    ft = ftp.tile([P, P], f32, tag="ft")
                    nc.vector.tensor_copy(out=ft[:], in_=reds[br][:])
                    for h in range(2):
                        mm = ps_mm.tile([P, 512], f32, tag="mm", space="PSUM")
                        nc.tensor.matmul(
                            out=mm[:], lhsT=ft[:], rhs=wt_t[:, h * 512:(h + 1) * 512],
                            start=True, stop=True)
                        nc.scalar.activation(
                            out=stage[:, 4 * h:4 * h + 4, BR_SEG[br], :],
                            in_=mm[:].rearrange("p (c d) -> p c d", c=4),
                            func=mybir.ActivationFunctionType.Relu,
                            scale=BR_SCALE[br])
                nc.sync.dma_start(
                    out=out[b, :, :],
                    in_=stage[:].rearrange("p a b d -> p (a b d)"))

    nc.compile()
    return nc


class _CoreRunner:
    def __init__(self, nc, device):
        import jax
        import concourse.mybir as mybir
        from concourse.bass2jax import (_bass_exec_p, install_neuronx_cc_hook,
                                        partition_id_tensor)
        install_neuronx_cc_hook()
        self.device = device
        partition_name = nc.partition_id_tensor.name if nc.partition_id_tensor else None
        in_names, out_names, out_avals = [], [], []
        for alloc in nc.m.functions[0].allocations:
            if not isinstance(alloc, mybir.MemoryLocationSet):
                continue
            name = alloc.memorylocations[0].name
            if alloc.kind == "ExternalInput":
                if name != partition_name:
                    in_names.append(name)
            elif alloc.kind == "ExternalOutput":
                out_names.append(name)
                out_avals.append(jax.core.ShapedArray(
                    tuple(alloc.tensor_shape), mybir.dt.np(alloc.dtype)))
        self.in_names, self.out_names, self.out_avals = in_names, out_names, out_avals
        all_in = list(in_names) + list(out_names)
        if partition_name is not None:
            all_in.append(partition_name)

        def _body(*args):
            operands = list(args)
            if partition_name is not None:
                operands.append(partition_id_tensor())
            return tuple(_bass_exec_p.bind(
                *operands, out_avals=tuple(out_avals), in_names=tuple(all_in),
                out_names=tuple(out_names), lowering_input_output_aliases=(),
                sim_require_finite=True, sim_require_nnan=True, nc=nc))

        self.fn = jax.jit(_body, keep_unused=True, device=device)

    def launch(self, in_map):
        import jax
        dev_in = [jax.device_put(np.asarray(in_map[n]), self.device)
                  for n in self.in_names]
        zeros = [jax.device_put(np.zeros(a.shape, a.dtype), self.device)
                 for a in self.out_avals]
        return self.fn(*dev_in, *zeros)


def _spot_check(out_flat, features, local_weight, nodes, neigh1, neigh2):
    """Recompute a few nodes on the host (fp32) and compare; guards against
    rare wedged-device garbage. Returns max rel err over the sample."""
    nf = np.asarray(nodes).reshape(-1)
    lw = np.asarray(local_weight).astype(np.float32)
    feats = np.asarray(features).astype(np.float32)
    n1, n2 = np.asarray(neigh1), np.asarray(neigh2)
    sample = [0, 1711, 4095, 8191]
    worst = 0.0
    denom = max(float(np.abs(out_flat).max()), 1e-6)
    for n in sample:
        v = int(nf[n])
        f_self = feats[v]
        f1 = feats[n1[v]].mean(axis=0)
        f2 = feats[n1[n2[v]]].mean(axis=(0, 1))
        pieces = [np.einsum('ckd,d->ck', lw, f) for f in (f1, f2, f_self)]
        exp = np.maximum(np.concatenate(pieces, axis=-1).reshape(-1), 0.0)
        err = float(np.abs(out_flat[n] - exp).max()) / denom
        worst = max(worst, err)
    return worst


def kernel(features, local_weight, nodes, neigh1, neigh2):
    import jax

    features = np.asarray(features)
    local_weight = np.asarray(local_weight)
    feat16 = features.astype(np.float16)
    WT = np.ascontiguousarray(
        np.transpose(local_weight, (2, 0, 1)).reshape(128, 1024)).astype(np.float32)

    in_maps, metas = [], []
    for c in range(8):
        im, meta = _pack_core(feat16, WT, nodes, neigh1, neigh2, c)
        in_maps.append(im)
        metas.append(meta)

    ncs = [build_core_program(m) for m in metas]
    devices = jax.devices()[:8]
    runners = [_CoreRunner(nc, d) for nc, d in zip(ncs, devices)]

    for attempt in range(3):
        futs = [r.launch(im) for r, im in zip(runners, in_maps)]
        jax.block_until_ready(futs)
        out = np.concatenate(
            [np.asarray(f[0]).astype(np.float32).reshape(1024, 3 * CK)
             for f in futs], axis=0)
        if _spot_check(out, features, local_weight, nodes, neigh1, neigh2) < 5e-3:
            break
    return out.reshape(1024, 8, 8, 3 * D)



# revision 5
# speedup vs baseline: 1.8949x; 1.8949x over previous
"""GNN message-passing encoder (nn_Encoder) for 8 Trainium2 NeuronCores.

Contract: kernel(**inputs) takes the FULL unsharded inputs of
reference.setup_inputs() and returns the FULL [1024, 8, 8, 384] float32
output. Internally the node batch (B*L = 8192 flat nodes) is sharded
1024 nodes per core across 8 cores; the feature table, neighbor lists
and weights are replicated.

Key layout trick: the host builds a query-independent re-layout of the
feature table, T2[v] = concat_j features_fp8[neigh1[v][j]] (100000 x
1280 fp8e4m3). A hop-2 sample (node n, j) then needs ONE 1280-byte
gather descriptor (u = neigh2[n][j] -> all 10 hop-1 rows of u) instead
of ten 256-byte ones, cutting gather-DMA time ~4.4x. The per-chunk 0/1
selection matmul that reduces gathered rows into feats^T [d, node]
reuses the SAME selection matrix for all 10 j-slices of a T2 row.

Per-core device program (node n = block*128 + p, 8 blocks):
  - hop2: per (block, table-segment) one SWDGE gather of ~320 T2 rows
    (fp8, int16 local indices); per 128-row chunk, 10 matmuls
    lhsT=rows[k,128] x rhs=M[k,w] accumulate feats2^T in PSUM.
  - hop1+self: per segment one gather of fp16 feature rows for all
    blocks; per-chunk matmuls split at block boundaries.
  - projection: feats^T (fp16, mean-scale folded into the PSUM->SBUF
    copy) x W^T [d, clips*dim] fp16 on the PE, fused ReLU on the
    scalar engine, one 768 KB fp16 store per block.
"""
import numpy as np

P = 128
NBLK = 8
S1, S2 = 10, 10
D = 128
CK = 1024
V = 100000
NSEG = 4
SEG = V // NSEG
N_CORE = 1024

BR_HOP1, BR_HOP2, BR_SELF = 0, 1, 2
BR_SCALE = {BR_HOP1: 0.1, BR_HOP2: 0.01, BR_SELF: 1.0}


def _f8dt():
    import ml_dtypes
    return ml_dtypes.float8_e4m3


_t2_cache = {}


def _get_t2(features16, neigh1):
    key = (id(features16), id(neigh1))
    if key not in _t2_cache:
        feat8 = np.asarray(features16).astype(_f8dt())
        t2 = feat8[np.asarray(neigh1).reshape(-1)].reshape(V, S1 * D)
        _t2_cache.clear()
        _t2_cache[key] = (features16, neigh1, np.ascontiguousarray(t2))
    return _t2_cache[key][2]


def _wrap_idx(idx):
    N = len(idx)
    w16 = idx.reshape(N // 16, 16).T.astype(np.int16)
    return np.tile(w16, (8, 1))


def _pad16(idx_list):
    n = len(idx_list)
    npad = (-n) % 16
    return np.array(list(idx_list) + [0] * npad, dtype=np.int16)


def _host_prep(nodes, neigh1, neigh2, core):
    nf = np.asarray(nodes).reshape(-1).astype(np.int64)
    shard = nf[core * N_CORE:(core + 1) * N_CORE]
    n1 = np.asarray(neigh1)[shard].astype(np.int64)   # [1024, 10]
    u2 = np.asarray(neigh2)[shard].astype(np.int64)   # [1024, 10]

    # ---- hop1+self: one gather per table segment covering all nodes ----
    br_nodes = np.concatenate([np.repeat(np.arange(N_CORE), S1),
                               np.arange(N_CORE)])
    br_rows = np.concatenate([n1.reshape(-1), shard])
    br_ids = np.concatenate([np.full(N_CORE * S1, BR_HOP1, dtype=np.int64),
                             np.full(N_CORE, BR_SELF, dtype=np.int64)])

    h1s_calls = []
    for s in range(NSEG):
        m = (br_rows // SEG) == s
        nd, rw, br = br_nodes[m], br_rows[m], br_ids[m]
        nvalid = len(nd)
        idx = _pad16((rw - s * SEG).tolist())
        chunks = []
        for c in range((nvalid + P - 1) // P):
            a, z = c * P, min((c + 1) * P, nvalid)
            # split [a, z) into runs of constant (branch, block)
            grp = br[a:z] * NBLK + nd[a:z] // P
            cuts = [0] + (np.nonzero(np.diff(grp))[0] + 1).tolist() + [z - a]
            pieces = []
            for ri in range(len(cuts) - 1):
                ra, rz = cuts[ri], cuts[ri + 1]
                brv = int(br[a + ra])
                blk = int(nd[a + ra]) // P
                cols = nd[a + ra:a + rz] - blk * P
                col0 = int(cols.min())
                w = int(cols.max()) - col0 + 1
                M = np.zeros((P, w), dtype=np.float16)
                M[np.arange(ra, rz), cols - col0] = 1.0
                pieces.append((brv, blk, ra, rz, col0, w, M))
            chunks.append((c, pieces))
        h1s_calls.append({"seg": s, "idx": idx, "nvalid": nvalid,
                          "chunks": chunks})

    # ---- hop2: one gather per (block, segment) from the T2 table ----
    hop2_calls = []
    for b in range(NBLK):
        ub = u2[b * P:(b + 1) * P]                   # [128, 10]
        nd_all = np.repeat(np.arange(P), S2)
        rw_all = ub.reshape(-1)
        for s in range(NSEG):
            m = (rw_all // SEG) == s
            nd, rw = nd_all[m], rw_all[m]
            nvalid = len(nd)
            idx = _pad16((rw - s * SEG).tolist())
            chunks = []
            for c in range((nvalid + P - 1) // P):
                a, z = c * P, min((c + 1) * P, nvalid)
                cols = nd[a:z]
                col0 = int(cols.min())
                w = int(cols.max()) - col0 + 1
                M = np.zeros((P, w), dtype=_f8dt())
                M[np.arange(z - a), cols - col0] = 1.0
                chunks.append((c, z - a, col0, w, M))
            hop2_calls.append({"b": b, "seg": s, "idx": idx,
                               "nvalid": nvalid, "chunks": chunks})
    return h1s_calls, hop2_calls


def _pack_core(features16, WT, nodes, neigh1, neigh2, core):
    t2 = _get_t2(features16, neigh1)
    h1s_calls, hop2_calls = _host_prep(nodes, neigh1, neigh2, core)

    idx_parts, m16_parts, m8_parts = [], [], []
    ioff = moff16 = moff8 = 0
    meta_h1s, meta_hop2 = [], []
    for cl in h1s_calls:
        w = _wrap_idx(cl["idx"])
        idx_parts.append(w)
        mchunks = []
        for (c, pieces) in cl["chunks"]:
            mp = []
            for (brv, blk, ra, rz, col0, wd, M) in pieces:
                m16_parts.append(M)
                mp.append((brv, blk, ra, rz, col0, wd, moff16))
                moff16 += wd
            mchunks.append((c, mp))
        meta_h1s.append({"seg": cl["seg"], "ioff": ioff,
                         "icols": w.shape[1], "nidx": len(cl["idx"]),
                         "chunks": mchunks})
        ioff += w.shape[1]
    for cl in hop2_calls:
        w = _wrap_idx(cl["idx"])
        idx_parts.append(w)
        mchunks = []
        for (c, k, col0, wd, M) in cl["chunks"]:
            m8_parts.append(M)
            mchunks.append((c, k, col0, wd, moff8))
            moff8 += wd
        meta_hop2.append({"b": cl["b"], "seg": cl["seg"], "ioff": ioff,
                          "icols": w.shape[1], "nidx": len(cl["idx"]),
                          "chunks": mchunks})
        ioff += w.shape[1]

    idx_all = np.ascontiguousarray(np.concatenate(idx_parts, axis=1))
    m16_all = np.ascontiguousarray(np.concatenate(m16_parts, axis=1))
    m8_all = np.ascontiguousarray(np.concatenate(m8_parts, axis=1))

    meta = {"h1s": meta_h1s, "hop2": meta_hop2,
            "icols": idx_all.shape[1], "c16": m16_all.shape[1],
            "c8": m8_all.shape[1]}
    in_map = {"feat": features16, "t2": t2,
              "wt": np.ascontiguousarray(WT.astype(np.float16)),
              "idxs": idx_all, "m16": m16_all, "m8": m8_all}
    return in_map, meta


def build_core_program(meta):
    import concourse.bacc as bacc
    import concourse.mybir as mybir
    from concourse.tile import TileContext
    from concourse.library_config import mlp

    f16, f32, i16 = mybir.dt.float16, mybir.dt.float32, mybir.dt.int16
    f8 = mybir.dt.float8e4
    Act = mybir.ActivationFunctionType

    nc = bacc.Bacc(num_swdge_queues=4)
    feat = nc.declare_dram_parameter("feat", [V, D], f16, isOutput=False)
    t2 = nc.declare_dram_parameter("t2", [V, S1 * D], f8, isOutput=False)
    wt = nc.declare_dram_parameter("wt", [D, CK], f16, isOutput=False)
    idxs = nc.declare_dram_parameter("idxs", [P, meta["icols"]], i16, isOutput=False)
    m16 = nc.declare_dram_parameter("m16", [P, meta["c16"]], f16, isOutput=False)
    m8 = nc.declare_dram_parameter("m8", [P, meta["c8"]], f8, isOutput=False)
    out = nc.declare_dram_parameter("out", [NBLK, P, 3 * CK], f16, isOutput=True)

    hop2_by_block = {}
    for cl in meta["hop2"]:
        hop2_by_block.setdefault(cl["b"], []).append(cl)

    with TileContext(nc) as tc:
        with (
            tc.tile_pool(name="const", bufs=1) as constp,
            tc.tile_pool(name="d16p", bufs=2) as d16p,
            tc.tile_pool(name="d8p", bufs=3) as d8p,
            tc.tile_pool(name="ftp", bufs=4) as ftp,
            tc.tile_pool(name="stp", bufs=2) as stp,
            tc.tile_pool(name="ps_red", bufs=1, space="PSUM") as ps_red,
            tc.tile_pool(name="ps_mm", bufs=2, space="PSUM") as ps_mm,
        ):
            nc.gpsimd.load_library(mlp)
            wt_t = constp.tile([P, CK], f16, tag="wt")
            nc.sync.dma_start(out=wt_t[:], in_=wt[:])
            it = constp.tile([P, meta["icols"]], i16, tag="it")
            nc.sync.dma_start(out=it[:], in_=idxs[:])
            mt16 = constp.tile([P, meta["c16"]], f16, tag="mt16")
            nc.sync.dma_start(out=mt16[:], in_=m16[:])
            mt8 = constp.tile([P, meta["c8"]], f8, tag="mt8")
            nc.sync.dma_start(out=mt8[:], in_=m8[:])
            zrhs = constp.tile([P, 4 * P], f16, tag="zrhs")
            nc.vector.memset(zrhs[:], 0.0)

            # PSUM is bank-granular (2 KB/partition): pack 4 blocks per
            # [128, 512] red tile; 3 branches x 2 groups = 6 banks + 2 mm.
            redg = {}
            for g in range(NBLK // 4):
                for br in range(3):
                    rt = ps_red.tile([P, 4 * P], f32, tag=f"red{br}_{g}", space="PSUM")
                    nc.tensor.matmul(out=rt[:], lhsT=zrhs[:, 0:P], rhs=zrhs[:],
                                     start=True, stop=False, skip_group_check=True)
                    redg[(br, g)] = rt

            def red_win(br, b, col0, wd):
                base = (b % 4) * P + col0
                return redg[(br, b // 4)][:, base:base + wd]

            # hop1 + self reductions
            for q, cl in enumerate(meta["h1s"]):
                npad = len_pad = cl["nidx"]
                nch = (npad + P - 1) // P
                dt16 = d16p.tile([P, nch, D], f16, tag="d16")
                nc.gpsimd.dma_gather(
                    dt16[:], feat[cl["seg"] * SEG:(cl["seg"] + 1) * SEG, :],
                    it[:, cl["ioff"]:cl["ioff"] + cl["icols"]],
                    npad, npad, D, single_packet=False, queue_num=q % 4)
                for (c, pieces) in cl["chunks"]:
                    for (brv, blk, ra, rz, col0, wd, mo) in pieces:
                        nc.tensor.matmul(
                            out=red_win(brv, blk, col0, wd),
                            lhsT=dt16[0:rz, c, :],
                            rhs=mt16[0:rz, mo:mo + wd],
                            start=False, stop=False, skip_group_check=True)

            # hop2 reductions + projection per group of 4 blocks
            for g in range(NBLK // 4):
                for b in range(4 * g, 4 * g + 4):
                    for q, cl in enumerate(hop2_by_block[b]):
                        npad = cl["nidx"]
                        if npad == 0:
                            continue
                        nch = (npad + P - 1) // P
                        dt8 = d8p.tile([P, nch, S1 * D], f8, tag="d8")
                        nc.gpsimd.dma_gather(
                            dt8[:], t2[cl["seg"] * SEG:(cl["seg"] + 1) * SEG, :],
                            it[:, cl["ioff"]:cl["ioff"] + cl["icols"]],
                            npad, npad, S1 * D, single_packet=False, queue_num=q % 4)
                        for (c, k, col0, wd, mo) in cl["chunks"]:
                            for j in range(S1):
                                nc.tensor.matmul(
                                    out=red_win(BR_HOP2, b, col0, wd),
                                    lhsT=dt8[0:k, c, j * D:(j + 1) * D],
                                    rhs=mt8[0:k, mo:mo + wd],
                                    start=False, stop=False, skip_group_check=True)

                for b in range(4 * g, 4 * g + 4):
                    stage = stp.tile([P, 8, 3, D], f16, tag="stage")
                    for br in range(3):
                        ft = ftp.tile([P, P], f16, tag="ft")
                        nc.scalar.activation(out=ft[:], in_=red_win(br, b, 0, P),
                                             func=Act.Copy, scale=BR_SCALE[br])
                        for h in range(2):
                            mm = ps_mm.tile([P, 512], f32, tag="mm", space="PSUM")
                            nc.tensor.matmul(
                                out=mm[:], lhsT=ft[:], rhs=wt_t[:, h * 512:(h + 1) * 512],
                                start=True, stop=True)
                            nc.scalar.activation(
                                out=stage[:, 4 * h:4 * h + 4, br, :],
                                in_=mm[:].rearrange("p (c d) -> p c d", c=4),
                                func=Act.Relu)
                    nc.sync.dma_start(
                        out=out[b, :, :],
                        in_=stage[:].rearrange("p a b d -> p (a b d)"))

    nc.compile()
    return nc


class _CoreRunner:
    def __init__(self, nc, device):
        import jax
        import concourse.mybir as mybir
        from concourse.bass2jax import (_bass_exec_p, install_neuronx_cc_hook,
                                        partition_id_tensor)
        install_neuronx_cc_hook()
        self.device = device
        partition_name = nc.partition_id_tensor.name if nc.partition_id_tensor else None
        in_names, out_names, out_avals = [], [], []
        for alloc in nc.m.functions[0].allocations:
            if not isinstance(alloc, mybir.MemoryLocationSet):
                continue
            name = alloc.memorylocations[0].name
            if alloc.kind == "ExternalInput":
                if name != partition_name:
                    in_names.append(name)
            elif alloc.kind == "ExternalOutput":
                out_names.append(name)
                out_avals.append(jax.core.ShapedArray(
                    tuple(alloc.tensor_shape), mybir.dt.np(alloc.dtype)))
        self.in_names, self.out_names, self.out_avals = in_names, out_names, out_avals
        all_in = list(in_names) + list(out_names)
        if partition_name is not None:
            all_in.append(partition_name)

        def _body(*args):
            operands = list(args)
            if partition_name is not None:
                operands.append(partition_id_tensor())
            return tuple(_bass_exec_p.bind(
                *operands, out_avals=tuple(out_avals), in_names=tuple(all_in),
                out_names=tuple(out_names), lowering_input_output_aliases=(),
                sim_require_finite=True, sim_require_nnan=True, nc=nc))

        self.fn = jax.jit(_body, keep_unused=True, device=device)

    def launch(self, in_map):
        import jax
        dev_in = [jax.device_put(np.asarray(in_map[n]), self.device)
                  for n in self.in_names]
        zeros = [jax.device_put(np.zeros(a.shape, a.dtype), self.device)
                 for a in self.out_avals]
        return self.fn(*dev_in, *zeros)


def _spot_check(out_flat, features, local_weight, nodes, neigh1, neigh2):
    """Recompute a few nodes on the host (fp32) and compare; guards against
    rare wedged-device garbage. Returns max rel err over the sample."""
    nf = np.asarray(nodes).reshape(-1)
    lw = np.asarray(local_weight).astype(np.float32)
    feats = np.asarray(features).astype(np.float32)
    n1, n2 = np.asarray(neigh1), np.asarray(neigh2)
    sample = [0, 1711, 4095, 8191]
    worst = 0.0
    denom = max(float(np.abs(out_flat).max()), 1e-6)
    for n in sample:
        v = int(nf[n])
        f_self = feats[v]
        f1 = feats[n1[v]].mean(axis=0)
        f2 = feats[n1[n2[v]]].mean(axis=(0, 1))
        pieces = [np.einsum('ckd,d->ck', lw, f) for f in (f1, f2, f_self)]
        exp = np.maximum(np.concatenate(pieces, axis=-1).reshape(-1), 0.0)
        err = float(np.abs(out_flat[n] - exp).max()) / denom
        worst = max(worst, err)
    return worst


def kernel(features, local_weight, nodes, neigh1, neigh2):
    import jax

    features = np.asarray(features)
    local_weight = np.asarray(local_weight)
    feat16 = features.astype(np.float16)
    WT = np.ascontiguousarray(
        np.transpose(local_weight, (2, 0, 1)).reshape(128, 1024)).astype(np.float32)

    in_maps, metas = [], []
    for c in range(8):
        im, meta = _pack_core(feat16, WT, nodes, neigh1, neigh2, c)
        in_maps.append(im)
        metas.append(meta)

    ncs = [build_core_program(m) for m in metas]
    devices = jax.devices()[:8]
    runners = [_CoreRunner(nc, d) for nc, d in zip(ncs, devices)]

    for attempt in range(3):
        futs = [r.launch(im) for r, im in zip(runners, in_maps)]
        jax.block_until_ready(futs)
        out = np.concatenate(
            [np.asarray(f[0]).astype(np.float32).reshape(1024, 3 * CK)
             for f in futs], axis=0)
        if _spot_check(out, features, local_weight, nodes, neigh1, neigh2) < 1.2e-2:
            break
    return out.reshape(1024, 8, 8, 3 * D)
